# revision 14
# baseline (speedup 1.0000x reference)
"""Multi-head causal self-attention with RoPE on 8 Trainium2 NeuronCores.

Sharding: (batch, head-group) data+tensor parallel. Core c handles batch
c//4 and heads [3*(c%4), 3*(c%4)+3). Each core runs fused
QKV-projection + RoPE + causal attention + output-projection and emits a
partial [S, D] output; the host sums the 4 head-group partials per batch.

Device-side layout choices:
  - x is fed transposed ([D, S]) so QKV matmuls contract d_model on
    partitions with x chunks stationary.
  - Q/K/V come out of the projection in [s, d] orientation. RoPE is
    applied there with 4 wide bf16 DVE ops per 2-seq-chunk group using
    host-built duplicated-cos [c|c] and signed-sin [-s|s] tables (the
    W_q/W_k rows are de-interleaved host-side so RoPE halves are
    contiguous 32-wide free-dim slices). Q/K are then transposed on the
    PE to [d, s] in stacked PAIRS ([128,128] per transpose, two heads at
    once); score matmuls read the halves via partition-offset operands.
  - Scores are computed transposed (S^T[k, q]) so the exp'd probability
    blocks feed the PV matmul directly. Softmax skips max-subtraction
    (scores*0.125 is O(5), exp safe in fp32), the denominator comes free
    from a ones-column appended to V, and causal masking is a post-exp
    multiply by constant 0/1 bf16 tiles (DVE 2x mode) on the two
    diagonal block-pairs only.
  - Output projection is interleaved into the attention loop (PE filler
    while ACT runs exp) one query-tile behind; PSUM budget is exactly 8
    banks: score pairs ring2 (4) + ctx ring2 (2) + out-proj ring2 (2).
"""

import numpy as np

import concourse.bass as bass
import concourse.tile as tile
from concourse import bacc, mybir
from concourse._compat import with_exitstack
from concourse.bass_utils import run_bass_kernel_spmd
from concourse.masks import make_identity

# Problem constants (hardcoded; kernel.py must be self-contained).
B = 2
S = 2048
D_MODEL = 768
NUM_HEADS = 12
HD = 64  # head dim
ROPE_THETA = 10000.0
MAX_SEQ_LEN = 2048

N_CORES = 8
HG = 3  # heads per core (12 heads / 4 head groups)
E = 3 * HG * HD  # 576: per-core qkv output rows
P = 128
NSC = S // P  # 16 seq chunks of 128
NKC = D_MODEL // P  # 6 d_model chunks of 128
QB = 512  # query block (free dim) in attention
NQT = S // QB  # 4 query tiles
VW = HD + 1  # V block width incl. ones column

F32 = mybir.dt.float32
BF16 = mybir.dt.bfloat16
MM = BF16
EXP = mybir.ActivationFunctionType.Exp
COPY = mybir.ActivationFunctionType.Copy


@with_exitstack
def emit_mhsa(ctx, tc, loop_m=1, phases="123", unroll=1):
    nc = tc.nc
    xT = nc.dram_tensor("xT", [D_MODEL, S], MM, kind="ExternalInput").ap()
    wqkvT = nc.dram_tensor("wqkvT", [D_MODEL, E], MM, kind="ExternalInput").ap()
    woT = nc.dram_tensor("woT", [HG * HD, D_MODEL], MM, kind="ExternalInput").ap()
    # duplicated [c|c] and signed [-s|s] rope tables, 64 wide, bf16
    cosg = nc.dram_tensor("cosg", [S, HD], MM, kind="ExternalInput").ap()
    sing = nc.dram_tensor("sing", [S, HD], MM, kind="ExternalInput").ap()
    out = nc.dram_tensor("out_partial", [S, D_MODEL], F32, kind="ExternalOutput").ap()

    const = ctx.enter_context(tc.tile_pool(name="const", bufs=1))
    persist = ctx.enter_context(tc.tile_pool(name="persist", bufs=1))

    # ---- constants & weights ----
    ident = const.tile([P, P], MM, tag="ident")
    make_identity(nc, ident[:])

    cos_sb = const.tile([P, NSC * HD], MM, tag="cos")
    sin_sb = const.tile([P, NSC * HD], MM, tag="sin")
    nc.sync.dma_start(
        cos_sb[:].rearrange("p (n f) -> p n f", f=HD),
        cosg.rearrange("(n p) f -> p n f", p=P),
    )
    nc.sync.dma_start(
        sin_sb[:].rearrange("p (n f) -> p n f", f=HD),
        sing.rearrange("(n p) f -> p n f", p=P),
    )

    w_sb = []
    for kc in range(NKC):
        w = const.tile([P, E], MM, tag=f"wqkv{kc}", name=f"wqkv{kc}")
        nc.sync.dma_start(w[:], wqkvT[kc * P : (kc + 1) * P, :])
        w_sb.append(w)
    wo0 = const.tile([P, D_MODEL], MM, tag="wo0")
    wo1 = const.tile([HD, D_MODEL], MM, tag="wo1")
    nc.sync.dma_start(wo0[:], woT[0:P, :])
    nc.sync.dma_start(wo1[:], woT[P : HG * HD, :])

    x_sb = []
    for kc in range(NKC):
        xt = const.tile([P, S], MM, tag=f"x{kc}", name=f"x{kc}")
        nc.sync.dma_start(xt[:, 0:256], xT[kc * P : (kc + 1) * P, 0:256])
        x_sb.append(xt)
    for kc in range(NKC):  # bulk of x after the head columns
        nc.sync.dma_start(x_sb[kc][:, 256:S], xT[kc * P : (kc + 1) * P, 256:S])

    # 0/1 bf16 causal masks for the two diagonal block-pairs of each
    # query tile: tile A covers diag blocks (j=0, j=1), tile B (j=2, j=3);
    # element (k, j*QB + q) keeps iff q >= 128*j + k.
    maskA = const.tile([P, 2 * QB], MM, tag="maskA")
    maskB = const.tile([P, 2 * QB], MM, tag="maskB")
    for t, (m, j0) in enumerate(((maskA, 0), (maskB, 2))):
        nc.gpsimd.memset(m[:], 1.0)
        for jj in range(2):
            nc.gpsimd.affine_select(
                out=m[:, jj * QB : (jj + 1) * QB],
                in_=m[:, jj * QB : (jj + 1) * QB],
                compare_op=mybir.AluOpType.is_ge, fill=0.0,
                base=-P * (j0 + jj), channel_multiplier=-1, pattern=[[1, QB]],
            )

    # ---- persistent intermediates ----
    # V (+ ones col) per head: head h block at h*NSC*VW, seq chunk sc at
    # +sc*VW. The ones columns are set ONCE here; V-copies only ever
    # write the [0:HD] slice of each block.
    v_sb = persist.tile([P, HG * NSC * VW], MM, tag="v")
    nc.gpsimd.memset(v_sb[:], 1.0)

    # Roped/transposed q,k: 4 pair-slots, pair p at cols [p*S,(p+1)*S),
    # pair contents on partitions [0:64] / [64:128]:
    #   pair0 = (q0, q1), pair1 = (k0, k1), pair2 = (q2, -), pair3 = (k2, -)
    # so each head's q and k sit at the SAME base partition (matmul
    # operands must share their base partition).
    qk_sb = persist.tile([P, 4 * S], MM, tag="qk")
    # ctx^T packed to match wo0/wo1 row packing: heads 0,1 in ctxA, head 2
    # in ctxB.
    ctxA = persist.tile([P, S], MM, tag="ctxA")
    ctxB = persist.tile([HD, S], MM, tag="ctxB")

    if loop_m > 1:  # timing builds only: repeat the compute body
        ctx.enter_context(tc.For_i(0, loop_m, 1))

    for _rep in range(unroll):
        emit_body(tc, phases, ident, cos_sb, sin_sb, w_sb, wo0, wo1, x_sb,
                  maskA, maskB, v_sb, qk_sb, ctxA, ctxB, out)


def q_ap(qk_sb, h, a, b):
    # q0 pair0 top, q1 pair0 bottom, q2 pair2 top
    pair, lo = ((0, 0), (0, 64), (2, 0))[h]
    return qk_sb[lo : lo + 64, pair * S + a : pair * S + b]


def k_ap(qk_sb, h, a, b):
    # k0 pair1 top, k1 pair1 bottom, k2 pair3 top
    pair, lo = ((1, 0), (1, 64), (3, 0))[h]
    return qk_sb[lo : lo + 64, pair * S + a : pair * S + b]


@with_exitstack
def emit_body(ctx, tc, phases, ident, cos_sb, sin_sb, w_sb, wo0, wo1, x_sb,
              maskA, maskB, v_sb, qk_sb, ctxA, ctxB, out):
    nc = tc.nc

    # ================= Phase 1: QKV + RoPE + Q/K transpose =================
    # Per 2-sc group: 24 qkv matmuls -> ACT copies to SBUF bf16 -> 4 wide
    # DVE RoPE ops -> 6 stacked-pair PE transposes (lagging one group) ->
    # ACT copies into qk_sb.
    if "1" in phases:
        with (
            tc.tile_pool(name="ps_qkv", bufs=2, space="PSUM") as ps_qkv,
            tc.tile_pool(name="ps_tr", bufs=2, space="PSUM") as ps_tr,
            tc.tile_pool(name="rope", bufs=2) as rope_pool,
        ):
            def emit_transposes(g, ro):
                # Per sc: 2 stacked-pair transposes (q0|q1, k0|k1) plus 2
                # singles (q2, k2 into pair tops); 2 scs per group. Copies
                # into qk_sb alternate ACT/DVE by sc parity to balance
                # engine load.
                for i in range(2):
                    sc = 2 * g + i
                    o = i * 384
                    pt = ps_tr.tile([P, 4 * P], MM, tag="pt", name="pt")
                    nc.tensor.transpose(pt[:, 0:128], ro[:, o : o + 128],
                                        ident[:])
                    nc.tensor.transpose(pt[:, 128:256], ro[:, o + 128 : o + 256],
                                        ident[:])
                    nc.tensor.transpose(pt[0:64, 256:384], ro[:, o + 256 : o + 320],
                                        ident[:])
                    nc.tensor.transpose(pt[0:64, 384:512], ro[:, o + 320 : o + 384],
                                        ident[:])
                    dst = qk_sb[:].rearrange("p (pair s) -> p pair s", pair=4)
                    d01 = dst[:, 0:2, sc * P : (sc + 1) * P]
                    s01 = pt[:, 0:256].rearrange("p (pair s) -> p pair s", pair=2)
                    d23 = dst[0:64, 2:4, sc * P : (sc + 1) * P]
                    s23 = pt[0:64, 256:512].rearrange("p (pair s) -> p pair s",
                                                      pair=2)
                    if sc % 2:
                        nc.scalar.copy(d01, s01)
                        nc.scalar.copy(d23, s23)
                    else:
                        nc.vector.tensor_copy(d01, s01)
                        nc.vector.tensor_copy(d23, s23)

            pend_tr = None  # transposes lag one group so PE never waits on RoPE
            for g in range(NSC // 2):
                qg = rope_pool.tile([P, 768], MM, tag="qg", name="qg")
                for i in range(2):
                    sc = 2 * g + i
                    pqk = ps_qkv.tile([P, 384], F32, tag="pqk")  # q|k
                    pv = ps_qkv.tile([P, HG * HD], F32, tag="pv")
                    for kc in range(NKC):
                        lhs = x_sb[kc][:, sc * P : (sc + 1) * P]
                        st, sp = kc == 0, kc == NKC - 1
                        nc.tensor.matmul(pqk[:], lhs, w_sb[kc][:, 0:384],
                                         start=st, stop=sp)
                        nc.tensor.matmul(pv[:], lhs, w_sb[kc][:, 384:576],
                                         start=st, stop=sp)
                    # q|k to SBUF bf16 for RoPE (ACT), V to its blocks (DVE)
                    nc.scalar.copy(qg[:, i * 384 : (i + 1) * 384], pqk[:])
                    v_dst = v_sb[:].rearrange("p (h n w) -> p h n w", h=HG, n=NSC)
                    nc.vector.tensor_copy(
                        v_dst[:, :, sc, 0:HD],
                        pv[:].rearrange("p (h w) -> p h w", h=HG),
                    )

                if pend_tr is not None:
                    emit_transposes(*pend_tr)

                # RoPE on the whole [128, 768] group: layout (g2, th6, f64)
                # with f = [ev-half 32 | od-half 32].
                sl = slice(2 * g * HD, (2 * g + 2) * HD)
                cos4 = (cos_sb[:, sl].rearrange("p (g f) -> p g f", g=2)
                        .unsqueeze(2).broadcast_to([P, 2, 6, HD]))
                sin4 = (sin_sb[:, sl].rearrange("p (g f) -> p g f", g=2)
                        .unsqueeze(2).broadcast_to([P, 2, 6, HD]))
                ro = rope_pool.tile([P, 768], MM, tag="ro", name="ro")
                t2 = rope_pool.tile([P, 768], MM, tag="t2", name="t2")
                q4 = qg[:].rearrange("p (g t f) -> p g t f", g=2, t=6)
                r4 = ro[:].rearrange("p (g t f) -> p g t f", g=2, t=6)
                t4 = t2[:].rearrange("p (g t f) -> p g t f", g=2, t=6)
                F = HD // 2
                nc.vector.tensor_mul(r4, q4, cos4)  # [c|c] both halves
                # swapped halves times [-s|s]
                nc.vector.tensor_mul(t4[:, :, :, 0:F], q4[:, :, :, F:HD],
                                     sin4[:, :, :, 0:F])
                nc.vector.tensor_mul(t4[:, :, :, F:HD], q4[:, :, :, 0:F],
                                     sin4[:, :, :, F:HD])
                nc.vector.tensor_add(ro[:], ro[:], t2[:])
                pend_tr = (g, ro)
            emit_transposes(*pend_tr)

    # ============ Phase 2+3: causal attention + output projection ==========
    if "2" not in phases:
        return
    do_p3 = "3" in phases
    with (
        tc.tile_pool(name="ps_s", bufs=2, space="PSUM") as ps_s_pool,
        tc.tile_pool(name="ps_ctx", bufs=2, space="PSUM") as ps_ctx_pool,
        tc.tile_pool(name="ps_o", bufs=2, space="PSUM") as ps_o_pool,
        tc.tile_pool(name="pp", bufs=8) as pp_pool,
        tc.tile_pool(name="norm", bufs=2) as norm_pool,
        tc.tile_pool(name="ob", bufs=2) as ob_pool,
    ):
        def emit_p3(qt, scs):
            # out-projection for seq chunks of query tile qt (ctx is
            # normalized); PE filler under the ACT-bound attention loop.
            for sc in scs:
                a_sl = ctxA[:, sc * P : (sc + 1) * P]
                b_sl = ctxB[:, sc * P : (sc + 1) * P]
                ob = ob_pool.tile([P, D_MODEL], F32, tag="ob", name="ob")
                for half in (0, 1):
                    c0 = half * 384
                    po = ps_o_pool.tile([P, 384], F32, tag="po", name="po")
                    nc.tensor.matmul(po[:], a_sl, wo0[:, c0 : c0 + 384],
                                     start=True, stop=False)
                    nc.tensor.matmul(po[:], b_sl, wo1[:, c0 : c0 + 384],
                                     start=False, stop=True)
                    nc.vector.tensor_copy(ob[:, c0 : c0 + 384], po[:])
                nc.sync.dma_start(out[sc * P : (sc + 1) * P, :], ob[:])

        for qt in range(NQT):
            nb = 4 * qt + 4
            npair = nb // 2
            for h in range(HG):
                qa = q_ap(qk_sb, h, qt * QB, (qt + 1) * QB)
                pctx = ps_ctx_pool.tile([VW, QB], F32, tag="pctx", name="pctx")
                pend = []  # exp'd block-pairs awaiting PV

                def emit_pv(psb, p):
                    for j in (0, 1):
                        kb = 2 * p + j
                        vb = h * NSC * VW + kb * VW
                        nc.tensor.matmul(
                            pctx[:], v_sb[:, vb : vb + VW],
                            psb[:, j * QB : (j + 1) * QB],
                            start=(kb == 0), stop=(kb == nb - 1),
                        )

                for p in range(npair):
                    pss = ps_s_pool.tile([P, 2 * QB], F32, tag="pss", name="pss")
                    for j in (0, 1):
                        nc.tensor.matmul(
                            pss[:, j * QB : (j + 1) * QB],
                            k_ap(qk_sb, h, (2 * p + j) * P, (2 * p + j + 1) * P),
                            qa, start=True, stop=True,
                        )
                    psb = pp_pool.tile([P, 2 * QB], MM, tag="psb", name="psb")
                    nc.scalar.activation(psb[:], pss[:], EXP, scale=0.125)
                    if p == npair - 2:  # diagonal pair A: 0/1 bf16 mask
                        nc.vector.tensor_mul(psb[:], psb[:], maskA[:])
                    elif p == npair - 1:  # diagonal pair B
                        nc.vector.tensor_mul(psb[:], psb[:], maskB[:])
                    pend.append((psb, p))
                    if len(pend) > 2:
                        emit_pv(*pend.pop(0))
                for args in pend:
                    emit_pv(*args)

                # normalize by the ones-row sum and write ctx^T
                rinv = norm_pool.tile([1, QB], F32, tag="rinv")
                nc.vector.reciprocal(rinv[0:1, :], pctx[HD : HD + 1, :])
                rbc = norm_pool.tile([HD, QB], F32, tag="rbc")
                nc.gpsimd.partition_broadcast(rbc[:], rinv[0:1, :])
                if h < 2:
                    dst = ctxA[h * HD : (h + 1) * HD, qt * QB : (qt + 1) * QB]
                else:
                    dst = ctxB[:, qt * QB : (qt + 1) * QB]
                nc.vector.tensor_mul(dst, pctx[0:HD, :], rbc[:])

                # interleave previous query-tile's output projection
                if do_p3 and qt > 0 and h < 2:
                    base = (qt - 1) * 4
                    emit_p3(qt - 1, range(base + 2 * h, base + 2 * h + 2))
        if do_p3:
            emit_p3(NQT - 1, range(12, 16))


_NC_CACHE = None


def build_nc(loop_m=1, phases="123", unroll=1):
    global _NC_CACHE
    key = (loop_m, phases, unroll)
    if _NC_CACHE is None or getattr(_NC_CACHE, "_key", None) != key:
        nc = bacc.Bacc("TRN2", target_bir_lowering=False, debug=False)
        with tile.TileContext(nc) as tc:
            emit_mhsa(tc, loop_m=loop_m, phases=phases, unroll=unroll)
        nc.compile()
        nc._key = key
        _NC_CACHE = nc
    return _NC_CACHE


def _rope_tables():
    powers = np.arange(0, HD, 2, dtype=np.float32) / np.float32(HD)
    freqs = (1.0 / (ROPE_THETA ** powers)).astype(np.float32)
    t = np.arange(MAX_SEQ_LEN, dtype=np.float32)
    ang = t[:, None] * freqs[None, :]
    return np.cos(ang).astype(np.float32), np.sin(ang).astype(np.float32)


def host_inputs(x, token_positions, W_qkv, W_o):
    """Build the 8 per-core input maps (shard + layout prep)."""
    import ml_dtypes
    mmdt = ml_dtypes.bfloat16

    x = np.asarray(x, dtype=np.float32)
    token_positions = np.asarray(token_positions)
    W_qkv = np.asarray(W_qkv, dtype=np.float32)
    W_o = np.asarray(W_o, dtype=np.float32)

    cos_t, sin_t = _rope_tables()
    # De-interleave head-dim rows of W_q/W_k so RoPE pairs become
    # contiguous 32-wide halves on device (dot products are invariant
    # to applying the same permutation to q and k).
    perm = np.concatenate([np.arange(0, HD, 2), np.arange(1, HD, 2)])
    Wq = W_qkv[0:D_MODEL].reshape(NUM_HEADS, HD, D_MODEL)[:, perm, :]
    Wk = W_qkv[D_MODEL : 2 * D_MODEL].reshape(NUM_HEADS, HD, D_MODEL)[:, perm, :]
    Wv = W_qkv[2 * D_MODEL : 3 * D_MODEL].reshape(NUM_HEADS, HD, D_MODEL)

    in_maps = []
    for c in range(N_CORES):
        b, g = divmod(c, 4)
        hs = slice(HG * g, HG * g + HG)
        Wq_c, Wk_c = Wq[hs], Wk[hs]  # [3, 64, 768] each
        # qk slot order q0 q1 k0 k1 q2 k2 (pairs transpose adjacently so
        # each head's q,k land at the same partition base); V unchanged.
        w_c = np.concatenate(
            [Wq_c[0], Wq_c[1], Wk_c[0], Wk_c[1], Wq_c[2], Wk_c[2],
             Wv[hs].reshape(HG * HD, D_MODEL)], axis=0)  # [576, 768]
        pos = np.asarray(token_positions[b], dtype=np.int64)
        cg, sg = cos_t[pos], sin_t[pos]  # [S, 32]
        cos_dup = np.concatenate([cg, cg], axis=1)  # [c|c]
        sin_sig = np.concatenate([-sg, sg], axis=1)  # [-s|s]
        in_maps.append({
            "xT": np.ascontiguousarray(x[b].T).astype(mmdt),
            "wqkvT": np.ascontiguousarray(w_c.T).astype(mmdt),
            "woT": np.ascontiguousarray(
                W_o[:, HG * g * HD : (HG * g + HG) * HD].T).astype(mmdt),
            "cosg": np.ascontiguousarray(cos_dup).astype(mmdt),
            "sing": np.ascontiguousarray(sin_sig).astype(mmdt),
        })
    return in_maps


def combine(partials):
    out = np.zeros((B, S, D_MODEL), dtype=np.float32)
    for c in range(N_CORES):
        out[c // 4] += partials[c]
    return out


def kernel(x, token_positions, W_qkv, W_o):
    nc = build_nc()
    in_maps = host_inputs(x, token_positions, W_qkv, W_o)
    res = run_bass_kernel_spmd(nc, in_maps, list(range(N_CORES)))
    return combine([res.results[c]["out_partial"] for c in range(N_CORES)])


# revision 17
# speedup vs baseline: 1.1462x; 1.1462x over previous
"""Multi-head causal self-attention with RoPE on 8 Trainium2 NeuronCores.

Sharding: (batch, head-group) data+tensor parallel. Core c handles batch
c//4 and heads [3*(c%4), 3*(c%4)+3). Each core runs fused
QKV-projection + RoPE + causal attention + output-projection and emits a
partial [S, D] output; the host sums the 4 head-group partials per batch.

Device-side layout choices:
  - x is fed transposed ([D, S]) so QKV matmuls contract d_model on
    partitions with x chunks stationary.
  - Q/K/V come out of the projection in [s, d] orientation. RoPE is
    applied there with 4 wide bf16 DVE ops per 2-seq-chunk group (2x DVE
    mode) using host-built duplicated-cos [c|c] and signed-sin [-s|s]
    bf16 tables; the W_q/W_k rows are de-interleaved host-side so RoPE
    halves are contiguous 32-wide free-dim slices. Q/K are then
    transposed on the PE to [d, s] for the score matmuls.
  - Scores are computed transposed (S^T[k, q]) so the exp'd probability
    blocks feed the PV matmul directly with no per-block transposes.
    Softmax skips the max-subtraction (scores*0.125 is O(5), exp is safe
    in fp32) and gets the denominator for free from a ones-column
    appended to V. Causal masking is a post-exp multiply by constant 0/1
    bf16 tiles (DVE 2x mode) on the two diagonal block-pairs only.
  - The V-block ones columns are initialized once outside the loop.
"""

import numpy as np

import concourse.bass as bass
import concourse.tile as tile
from concourse import bacc, mybir
from concourse._compat import with_exitstack
from concourse.bass_utils import run_bass_kernel_spmd
from concourse.masks import make_identity

# Problem constants (hardcoded; kernel.py must be self-contained).
B = 2
S = 2048
D_MODEL = 768
NUM_HEADS = 12
HD = 64  # head dim
ROPE_THETA = 10000.0
MAX_SEQ_LEN = 2048

N_CORES = 8
HG = 3  # heads per core (12 heads / 4 head groups)
E = 3 * HG * HD  # 576: per-core qkv output rows
P = 128
NSC = S // P  # 16 seq chunks of 128
NKC = D_MODEL // P  # 6 d_model chunks of 128
QB = 512  # query block (free dim) in attention
NQT = S // QB  # 4 query tiles
VW = HD + 1  # V block width incl. ones column

F32 = mybir.dt.float32
BF16 = mybir.dt.bfloat16
MM = BF16
EXP = mybir.ActivationFunctionType.Exp


@with_exitstack
def emit_mhsa(ctx, tc, loop_m=1, phases="123", unroll=1, barrier=False):
    nc = tc.nc
    xT = nc.dram_tensor("xT", [D_MODEL, S], MM, kind="ExternalInput").ap()
    wqkvT = nc.dram_tensor("wqkvT", [D_MODEL, E], MM, kind="ExternalInput").ap()
    woT = nc.dram_tensor("woT", [HG * HD, D_MODEL], MM, kind="ExternalInput").ap()
    # duplicated [c|c] and signed [-s|s] rope tables, 64 wide, bf16
    cosg = nc.dram_tensor("cosg", [S, HD], MM, kind="ExternalInput").ap()
    sing = nc.dram_tensor("sing", [S, HD], MM, kind="ExternalInput").ap()
    out = nc.dram_tensor("out_partial", [S, D_MODEL], F32, kind="ExternalOutput").ap()

    const = ctx.enter_context(tc.tile_pool(name="const", bufs=1))
    persist = ctx.enter_context(tc.tile_pool(name="persist", bufs=1))

    # ---- constants & weights ----
    ident = const.tile([P, P], MM, tag="ident")
    make_identity(nc, ident[:])

    cos_sb = const.tile([P, NSC * HD], MM, tag="cos")
    sin_sb = const.tile([P, NSC * HD], MM, tag="sin")
    nc.sync.dma_start(
        cos_sb[:].rearrange("p (n f) -> p n f", f=HD),
        cosg.rearrange("(n p) f -> p n f", p=P),
    )
    nc.sync.dma_start(
        sin_sb[:].rearrange("p (n f) -> p n f", f=HD),
        sing.rearrange("(n p) f -> p n f", p=P),
    )

    w_sb = []
    for kc in range(NKC):
        w = const.tile([P, E], MM, tag=f"wqkv{kc}", name=f"wqkv{kc}")
        nc.sync.dma_start(w[:], wqkvT[kc * P : (kc + 1) * P, :])
        w_sb.append(w)
    wo0 = const.tile([P, D_MODEL], MM, tag="wo0")
    wo1 = const.tile([HD, D_MODEL], MM, tag="wo1")
    nc.sync.dma_start(wo0[:], woT[0:P, :])
    nc.sync.dma_start(wo1[:], woT[P : HG * HD, :])

    x_sb = []
    for kc in range(NKC):
        xt = const.tile([P, S], MM, tag=f"x{kc}", name=f"x{kc}")
        nc.sync.dma_start(xt[:, 0:256], xT[kc * P : (kc + 1) * P, 0:256])
        x_sb.append(xt)
    for kc in range(NKC):  # bulk of x after the head columns
        nc.sync.dma_start(x_sb[kc][:, 256:S], xT[kc * P : (kc + 1) * P, 256:S])

    # 0/1 bf16 causal masks for the two diagonal block-pairs of each
    # query tile: tile A covers diag blocks (j=0,1), tile B (j=2,3);
    # element (k, jj*QB + q) keeps iff q >= 128*j + k.
    maskA = const.tile([P, 2 * QB], MM, tag="maskA")
    maskB = const.tile([P, 2 * QB], MM, tag="maskB")
    for m, j0 in ((maskA, 0), (maskB, 2)):
        nc.gpsimd.memset(m[:], 1.0)
        for jj in range(2):
            nc.gpsimd.affine_select(
                out=m[:, jj * QB : (jj + 1) * QB],
                in_=m[:, jj * QB : (jj + 1) * QB],
                compare_op=mybir.AluOpType.is_ge, fill=0.0,
                base=-P * (j0 + jj), channel_multiplier=-1, pattern=[[1, QB]],
            )

    # ---- persistent intermediates (all at base partition 0) ----
    # V (+ ones col) per head: head h block at h*NSC*VW, seq chunk sc at
    # +sc*VW. Ones columns are set ONCE here; V-copies only write [0:HD].
    v_sb = persist.tile([P, HG * NSC * VW], MM, tag="v")
    nc.gpsimd.memset(v_sb[:], 1.0)

    # All 6 roped/transposed q,k heads side by side: slot i at cols
    # [i*S, (i+1)*S) in slot order q0 q1 q2 k0 k1 k2.
    qk_sb = persist.tile([HD, 6 * S], MM, tag="qk")
    # ctx^T packed to match wo0/wo1 row packing: heads 0,1 in ctxA, head 2
    # in ctxB.
    ctxA = persist.tile([P, S], MM, tag="ctxA")
    ctxB = persist.tile([HD, S], MM, tag="ctxB")

    if loop_m > 1:  # timing builds only: repeat the compute body
        ctx.enter_context(tc.For_i(0, loop_m, 1))

    for _rep in range(unroll):
        emit_body(tc, phases, ident, cos_sb, sin_sb, w_sb, wo0, wo1, x_sb,
                  maskA, maskB, v_sb, qk_sb, ctxA, ctxB, out)


@with_exitstack
def emit_body(ctx, tc, phases, ident, cos_sb, sin_sb, w_sb, wo0, wo1, x_sb,
              maskA, maskB, v_sb, qk_sb, ctxA, ctxB, out):
    nc = tc.nc

    def q_sb_ap(h, a, b):
        return qk_sb[:, h * S + a : h * S + b]

    def k_sb_ap(h, a, b):
        return qk_sb[:, (HG + h) * S + a : (HG + h) * S + b]

    # ================= Phase 1: QKV + RoPE + Q/K transpose =================
    if "1" in phases:
        with (
            tc.tile_pool(name="ps_qkv", bufs=3, space="PSUM") as ps_qkv,
            tc.tile_pool(name="ps_tr", bufs=2, space="PSUM") as ps_tr,
            tc.tile_pool(name="rope", bufs=2) as rope_pool,
        ):
            def emit_transposes(g, ro):
                # 6 per-head transposes per sc into one PSUM bank, one
                # strided copy out; 2 scs per group.
                for i in range(2):
                    sc = 2 * g + i
                    pt = ps_tr.tile([HD, 6 * P], MM, tag="pt", name="pt")
                    for sl in range(6):
                        nc.tensor.transpose(
                            pt[:, sl * P : (sl + 1) * P],
                            ro[:, i * 384 + sl * HD : i * 384 + (sl + 1) * HD],
                            ident[:],
                        )
                    dst = qk_sb[:].rearrange("p (slot s) -> p slot s", slot=6)
                    nc.scalar.copy(
                        dst[:, :, sc * P : (sc + 1) * P],
                        pt[:].rearrange("p (slot s) -> p slot s", slot=6),
                    )

            pend_tr = None  # transposes lag one group so PE never waits on RoPE
            for g in range(NSC // 2):
                qg = rope_pool.tile([P, 768], MM, tag="qg", name="qg")
                for i in range(2):
                    sc = 2 * g + i
                    pqk = ps_qkv.tile([P, 384], F32, tag="pqk")  # q|k
                    pv = ps_qkv.tile([P, HG * HD], F32, tag="pv")
                    for kc in range(NKC):
                        lhs = x_sb[kc][:, sc * P : (sc + 1) * P]
                        st, sp = kc == 0, kc == NKC - 1
                        nc.tensor.matmul(pqk[:], lhs, w_sb[kc][:, 0:384],
                                         start=st, stop=sp)
                        nc.tensor.matmul(pv[:], lhs, w_sb[kc][:, 384:576],
                                         start=st, stop=sp)
                    # q|k to SBUF bf16 for RoPE (ACT); V to its blocks (DVE)
                    nc.scalar.copy(qg[:, i * 384 : (i + 1) * 384], pqk[:])
                    v_dst = v_sb[:].rearrange("p (h n w) -> p h n w", h=HG, n=NSC)
                    nc.vector.tensor_copy(
                        v_dst[:, :, sc, 0:HD],
                        pv[:].rearrange("p (h w) -> p h w", h=HG),
                    )

                if pend_tr is not None:
                    emit_transposes(*pend_tr)

                # RoPE on the whole [128, 768] group: layout (g2, th6, f64)
                # with f = [ev-half 32 | od-half 32]; all-bf16 ops (2x DVE).
                sl2 = slice(2 * g * HD, (2 * g + 2) * HD)
                cos4 = (cos_sb[:, sl2].rearrange("p (g f) -> p g f", g=2)
                        .unsqueeze(2).broadcast_to([P, 2, 6, HD]))
                sin4 = (sin_sb[:, sl2].rearrange("p (g f) -> p g f", g=2)
                        .unsqueeze(2).broadcast_to([P, 2, 6, HD]))
                ro = rope_pool.tile([P, 768], MM, tag="ro", name="ro")
                t2 = rope_pool.tile([P, 768], MM, tag="t2", name="t2")
                q4 = qg[:].rearrange("p (g t f) -> p g t f", g=2, t=6)
                r4 = ro[:].rearrange("p (g t f) -> p g t f", g=2, t=6)
                t4 = t2[:].rearrange("p (g t f) -> p g t f", g=2, t=6)
                F = HD // 2
                nc.vector.tensor_mul(r4, q4, cos4)  # [c|c] both halves
                nc.vector.tensor_mul(t4[:, :, :, 0:F], q4[:, :, :, F:HD],
                                     sin4[:, :, :, 0:F])
                nc.vector.tensor_mul(t4[:, :, :, F:HD], q4[:, :, :, 0:F],
                                     sin4[:, :, :, F:HD])
                nc.vector.tensor_add(ro[:], ro[:], t2[:])
                pend_tr = (g, ro)
            emit_transposes(*pend_tr)

    # ================= Phase 2: causal attention (S^T form) =================
    if "2" not in phases:
        return
    with (
        tc.tile_pool(name="ps_s", bufs=3, space="PSUM") as ps_s_pool,
        tc.tile_pool(name="ps_ctx", bufs=2, space="PSUM") as ps_ctx_pool,
        tc.tile_pool(name="pp", bufs=8) as pp_pool,
        tc.tile_pool(name="norm", bufs=2) as norm_pool,
    ):
        # qt-outer so phase 3 can start on early seq chunks while attention
        # continues; PV matmuls lag the score matmuls so the in-order PE
        # never stalls on the ACT-exp chain.
        for qt in range(NQT):
            nb = 4 * qt + 4
            npair = nb // 2
            for h in range(HG):
                qa = q_sb_ap(h, qt * QB, (qt + 1) * QB)
                pctx = ps_ctx_pool.tile([VW, QB], F32, tag="pctx", name="pctx")
                pend = []  # exp'd block PAIRS awaiting PV

                def emit_pv(psb, p):
                    for j in (0, 1):
                        kb = 2 * p + j
                        vb = h * NSC * VW + kb * VW
                        nc.tensor.matmul(
                            pctx[:], v_sb[:, vb : vb + VW],
                            psb[:, j * QB : (j + 1) * QB],
                            start=(kb == 0), stop=(kb == nb - 1),
                        )

                for p in range(npair):
                    pss = ps_s_pool.tile([P, 2 * QB], F32, tag="pss", name="pss")
                    for j in (0, 1):
                        nc.tensor.matmul(
                            pss[:, j * QB : (j + 1) * QB],
                            k_sb_ap(h, (2 * p + j) * P, (2 * p + j + 1) * P),
                            qa, start=True, stop=True,
                        )
                    psb = pp_pool.tile([P, 2 * QB], MM, tag="psb", name="psb")
                    nc.scalar.activation(psb[:], pss[:], EXP, scale=0.125)
                    # diagonal pairs: post-exp 0/1 bf16 mask (DVE 2x mode)
                    if p == npair - 2:
                        nc.vector.tensor_mul(psb[:], psb[:], maskA[:])
                    elif p == npair - 1:
                        nc.vector.tensor_mul(psb[:], psb[:], maskB[:])
                    pend.append((psb, p))
                    if len(pend) > 2:
                        emit_pv(*pend.pop(0))
                for args in pend:
                    emit_pv(*args)
                # normalize by the ones-row sum and write ctx^T
                rinv = norm_pool.tile([1, QB], F32, tag="rinv")
                nc.vector.reciprocal(rinv[0:1, :], pctx[HD : HD + 1, :])
                rbc = norm_pool.tile([HD, QB], F32, tag="rbc")
                nc.gpsimd.partition_broadcast(rbc[:], rinv[0:1, :])
                if h < 2:
                    dst = ctxA[h * HD : (h + 1) * HD, qt * QB : (qt + 1) * QB]
                else:
                    dst = ctxB[:, qt * QB : (qt + 1) * QB]
                nc.vector.tensor_mul(dst, pctx[0:HD, :], rbc[:])

    # ================= Phase 3: output projection =================
    if "3" not in phases:
        return
    with (
        tc.tile_pool(name="ps_o", bufs=3, space="PSUM") as ps_o_pool,
        tc.tile_pool(name="ob", bufs=4) as ob_pool,
    ):
        for sc in range(NSC):
            po = ps_o_pool.tile([P, D_MODEL], F32, tag="po", name="po")
            a_sl = ctxA[:, sc * P : (sc + 1) * P]
            b_sl = ctxB[:, sc * P : (sc + 1) * P]
            nc.tensor.matmul(po[:, 0:512], a_sl, wo0[:, 0:512], start=True, stop=False)
            nc.tensor.matmul(po[:, 0:512], b_sl, wo1[:, 0:512], start=False, stop=True)
            nc.tensor.matmul(po[:, 512:768], a_sl, wo0[:, 512:768], start=True, stop=False)
            nc.tensor.matmul(po[:, 512:768], b_sl, wo1[:, 512:768], start=False, stop=True)
            ob = ob_pool.tile([P, D_MODEL], F32, tag="ob")
            # alternate the PSUM->SBUF copy between ACT and DVE
            if sc % 2:
                nc.scalar.copy(ob[:], po[:])
            else:
                nc.vector.tensor_copy(ob[:], po[:])
            # split the store for DMA-queue parallelism at the kernel tail
            nc.sync.dma_start(out[sc * P : (sc + 1) * P, 0:384], ob[:, 0:384])
            nc.sync.dma_start(out[sc * P : (sc + 1) * P, 384:768], ob[:, 384:768])


_NC_CACHE = None


def build_nc(loop_m=1, phases="123", unroll=1, barrier=False):
    global _NC_CACHE
    key = (loop_m, phases, unroll, barrier)
    if _NC_CACHE is None or getattr(_NC_CACHE, "_key", None) != key:
        nc = bacc.Bacc("TRN2", target_bir_lowering=False, debug=False)
        with tile.TileContext(nc) as tc:
            emit_mhsa(tc, loop_m=loop_m, phases=phases, unroll=unroll,
                      barrier=barrier)
        nc.compile()
        nc._key = key
        _NC_CACHE = nc
    return _NC_CACHE


def _rope_tables():
    powers = np.arange(0, HD, 2, dtype=np.float32) / np.float32(HD)
    freqs = (1.0 / (ROPE_THETA ** powers)).astype(np.float32)
    t = np.arange(MAX_SEQ_LEN, dtype=np.float32)
    ang = t[:, None] * freqs[None, :]
    return np.cos(ang).astype(np.float32), np.sin(ang).astype(np.float32)


def host_inputs(x, token_positions, W_qkv, W_o):
    """Build the 8 per-core input maps (shard + layout prep)."""
    import ml_dtypes
    mmdt = ml_dtypes.bfloat16

    x = np.asarray(x, dtype=np.float32)
    token_positions = np.asarray(token_positions)
    W_qkv = np.asarray(W_qkv, dtype=np.float32)
    W_o = np.asarray(W_o, dtype=np.float32)

    cos_t, sin_t = _rope_tables()
    # De-interleave head-dim rows of W_q/W_k so RoPE pairs become
    # contiguous 32-wide halves on device (dot products are invariant
    # to applying the same permutation to q and k).
    perm = np.concatenate([np.arange(0, HD, 2), np.arange(1, HD, 2)])
    Wq = W_qkv[0:D_MODEL].reshape(NUM_HEADS, HD, D_MODEL)[:, perm, :]
    Wk = W_qkv[D_MODEL : 2 * D_MODEL].reshape(NUM_HEADS, HD, D_MODEL)[:, perm, :]
    Wv = W_qkv[2 * D_MODEL : 3 * D_MODEL].reshape(NUM_HEADS, HD, D_MODEL)

    in_maps = []
    for c in range(N_CORES):
        b, g = divmod(c, 4)
        hs = slice(HG * g, HG * g + HG)
        w_c = np.concatenate(
            [Wq[hs].reshape(HG * HD, D_MODEL),
             Wk[hs].reshape(HG * HD, D_MODEL),
             Wv[hs].reshape(HG * HD, D_MODEL)], axis=0)  # [576, 768]
        pos = np.asarray(token_positions[b], dtype=np.int64)
        cg, sg = cos_t[pos], sin_t[pos]  # [S, 32]
        cos_dup = np.concatenate([cg, cg], axis=1)  # [c|c]
        sin_sig = np.concatenate([-sg, sg], axis=1)  # [-s|s]
        in_maps.append({
            "xT": np.ascontiguousarray(x[b].T).astype(mmdt),
            "wqkvT": np.ascontiguousarray(w_c.T).astype(mmdt),
            "woT": np.ascontiguousarray(
                W_o[:, HG * g * HD : (HG * g + HG) * HD].T).astype(mmdt),
            "cosg": np.ascontiguousarray(cos_dup).astype(mmdt),
            "sing": np.ascontiguousarray(sin_sig).astype(mmdt),
        })
    return in_maps


def combine(partials):
    out = np.zeros((B, S, D_MODEL), dtype=np.float32)
    for c in range(N_CORES):
        out[c // 4] += partials[c]
    return out


def kernel(x, token_positions, W_qkv, W_o):
    nc = build_nc()
    in_maps = host_inputs(x, token_positions, W_qkv, W_o)
    res = run_bass_kernel_spmd(nc, in_maps, list(range(N_CORES)))
    return combine([res.results[c]["out_partial"] for c in range(N_CORES)])
